# revision 2
# baseline (speedup 1.0000x reference)
"""Trainium2 Bass kernel v2 for nn_Attention_71210557768228.

Single-layer non-causal attention with RoPE:
  x:[4,2048,1024] -> qkv (no bias) -> RoPE(q,k) -> softmax(q k^T / 8) v -> proj + bias

Sharding across 8 NeuronCores: core = (batch b in 0..3, head-group g in 0..1).
Each core processes one batch and 8 of the 16 heads end-to-end and produces a
partial projection output [2048, 1024]; the host sums the two head-group
partials per batch and adds the bias.

v2 layout choices (cost-model-driven):
  - q/k/v and output projections in bf16 (fp8 weights measured too lossy).
  - RoPE on DVE (two PSUM muls + quadrant shuffle) with the final add on the
    GPSIMD/Pool engine writing fp8e4m3; an SBUF->SBUF DMA repacks each f-tile
    into the DoubleRow layout [32p, 2u, T] per head parity.
  - scores in fp8 DoubleRow (d=64 contraction as 32 partitions x 2): half-rate
    rows double the score throughput; error budget ~1% of output scale.
  - softmax exp split between ScalarE (native Exp, scale=0.125 folded) and DVE
    (Schraudolph exp2 bit-trick: one tensor_scalar to int16 == bf16 bits).
  - attn@v in bf16 *flipped*: out [128 i-chunk, 65] with pexp as the
    stationary operand, so cost scales with the 65-wide free dim, the softmax
    denominator arrives via a ones column of V, and normalization is a cheap
    per-partition-scalar multiply. PE transposes (identity matmul) restore the
    [dv, t] layout for the output projection; a cross-partition-base copy
    lands it in y2 without any DMA.
  - PSUM: pq(2) + sp(2) + yu(2) + tr(2) = 8 banks.
"""

import sys

import numpy as np
import ml_dtypes

_REPO = "/opt/trn_rl_repo"
if _REPO not in sys.path:
    sys.path.insert(0, _REPO)

import concourse.bass as bass
import concourse.bacc as bacc
import concourse.mybir as mybir
from concourse.bass import ts
from concourse.tile import TileContext

F32 = mybir.dt.float32
BF16 = mybir.dt.bfloat16
FP8 = mybir.dt.float8e4
I16 = mybir.dt.int16
DR = mybir.MatmulPerfMode.DoubleRow

DIM, H, D = 1024, 16, 64
B, T = 4, 2048
G = 2                 # head groups (cores per batch)
HG = H // G           # heads per group = 8
DV = HG * D           # per-core v width = 512
N_CORES = 8

CT = DIM // 128       # 8 contraction tiles
NJT = T // 128        # 16 key-token tiles
NIT = T // 512        # 4 query chunks of 512
SWAP16 = [(i + 16) % 32 for i in range(32)]

# Schraudolph bf16-bits-of-2^t constants: i16 = round(s*0.125*log2e*128 +
# (127 + C)*128); C chosen so the mean ratio vs exact exp is 1.0 over the
# score distribution (DVE and ScalarE chunks mix in one softmax, so a bias
# would NOT cancel in the ratio).
SCHR_A = 0.125 * 1.4426950408889634 * 128.0
SCHR_B = (127.0 - 0.0575) * 128.0


# ---------------------------------------------------------------- host prep

def _perm64():
    perm = np.zeros(64, dtype=np.int64)
    for q in range(2):
        for i in range(16):
            perm[32 * q + i] = 32 * q + 2 * i
            perm[32 * q + 16 + i] = 32 * q + 2 * i + 1
    return perm


def _cos_sin_tiles(freqs):
    """cosR, sinX [128, T] fp32 (rows replicate with period 64)."""
    perm = _perm64()
    cos = np.cos(freqs)            # [T, 64]
    sin = np.sin(freqs)
    cos64 = np.ascontiguousarray(cos[:, perm].T)     # [64, T]
    sinX64 = np.empty_like(cos64)
    for r in range(64):
        q, i = r // 32, r % 32
        sw = 32 * q + ((i + 16) % 32)
        sign = 1.0 if i < 16 else -1.0
        sinX64[r] = sign * sin[:, perm[sw]]
    cosR = np.concatenate([cos64, cos64], axis=0).astype(np.float32)
    sinX = np.concatenate([sinX64, sinX64], axis=0).astype(np.float32)
    return cosR, sinX


# ---------------------------------------------------------------- bass build

def build_nc(exp_dve=(2, 5), px_bufs=34, rope_bufs=3, r8_bufs=4, osb_bufs=3,
             y2t_bufs=2, debug=False):
    nc = bacc.Bacc("TRN2", target_bir_lowering=False)

    xT_d = nc.dram_tensor("xT", (DIM, T), BF16, kind="ExternalInput")
    wqk_d = nc.dram_tensor("wqkT", (DIM, 2 * DV), BF16, kind="ExternalInput")
    wv_d = nc.dram_tensor("wvT", (DIM, DV), BF16, kind="ExternalInput")
    wp_d = nc.dram_tensor("wpT", (DV, DIM), BF16, kind="ExternalInput")
    cos_d = nc.dram_tensor("cosR", (128, T), F32, kind="ExternalInput")
    sin_d = nc.dram_tensor("sinX", (128, T), F32, kind="ExternalInput")
    id_d = nc.dram_tensor("ident", (128, 128), BF16, kind="ExternalInput")
    out_d = nc.dram_tensor("out_part", (T, DIM), F32, kind="ExternalOutput")
    if debug:
        dbg_qk8 = nc.dram_tensor("dbg_qk8", (64, 2 * DV // 128, 2, T), FP8,
                                 kind="ExternalOutput")
        dbg_v16 = nc.dram_tensor("dbg_v16", (128, NJT, HG, 66), BF16,
                                 kind="ExternalOutput")
        dbg_y2 = nc.dram_tensor("dbg_y2", (128, DV // 128, T), BF16,
                                kind="ExternalOutput")
        dbg_sp = nc.dram_tensor("dbg_sp", (128, 512), F32,
                                kind="ExternalOutput")
        dbg_px = nc.dram_tensor("dbg_px", (128, 512), BF16,
                                kind="ExternalOutput")
        dbg_yu = nc.dram_tensor("dbg_yu", (128, 4, 65), F32,
                                kind="ExternalOutput")
        dbg_y2t = nc.dram_tensor("dbg_y2t", (128, 4, 64), BF16,
                                 kind="ExternalOutput")

    exp_cnt = [0]

    with TileContext(nc) as tc:
        with tc.tile_pool(name="const", bufs=1) as cpool:
            wqk_sb = cpool.tile([128, CT, 2 * DV], BF16)
            wv_sb = cpool.tile([128, CT, DV], BF16)
            wp_sb = cpool.tile([128, DV // 128, DIM], BF16)
            cos_sb = cpool.tile([128, T], F32)
            sin_sb = cpool.tile([128, T], F32)
            id_sb = cpool.tile([128, 128], BF16)
            # DoubleRow q/k store: [32*hp + pp, slot(ft), u, t] fp8
            qk8 = cpool.tile([64, 2 * DV // 128, 2, T], FP8)
            # v in bf16: [j, jt, h, 66] with ones at col 64 (rowsum trick)
            v16 = cpool.tile([128, NJT, HG, 66], BF16)
            y2_sb = cpool.tile([128, DV // 128, T], BF16)

            nc.vector.memset(v16[:, :, :, 64:66], 0.0)
            nc.vector.memset(v16[:, :, :, 64:65], 1.0)

            with tc.tile_pool(name="pA", bufs=1) as apool, \
                 tc.tile_pool(name="ps", bufs=1, space="PSUM") as psum, \
                 tc.tile_pool(name="rope", bufs=rope_bufs) as rpool, \
                 tc.tile_pool(name="r8p", bufs=r8_bufs) as r8pool, \
                 tc.tile_pool(name="px", bufs=px_bufs) as pxpool, \
                 tc.tile_pool(name="nrm", bufs=2) as npool, \
                 tc.tile_pool(name="y2t", bufs=y2t_bufs) as ypool, \
                 tc.tile_pool(name="osb", bufs=osb_bufs) as opool:
                xT_sb = apool.tile([128, CT, T], BF16)
                nc.scalar.dma_start(cos_sb[:], cos_d[:])
                nc.scalar.dma_start(sin_sb[:], sin_d[:])
                nc.scalar.dma_start(id_sb[:], id_d[:])
                for ct in range(CT):
                    nc.sync.dma_start(xT_sb[:, ct, :], xT_d[ts(ct, 128), :])
                    nc.scalar.dma_start(wqk_sb[:, ct, :], wqk_d[ts(ct, 128), :])
                for ct in range(CT):
                    nc.scalar.dma_start(wv_sb[:, ct, :], wv_d[ts(ct, 128), :])
                for d4 in range(DV // 128):
                    nc.scalar.dma_start(wp_sb[:, d4, :], wp_d[ts(d4, 128), :])

                rq_cnt = [0]

                def qk_tile(ft, tq):
                    r8c = r8pool.tile([128, 512], FP8, tag="r8c",
                                      name=f"r8_{ft}_{tq}")
                    pq = psum.tile([128, 512], F32, tag="pq", bufs=2, name="pq")
                    for ct in range(CT):
                        nc.tensor.matmul(
                            pq,
                            lhsT=wqk_sb[:, ct, ts(ft, 128)],
                            rhs=xT_sb[:, ct, ts(tq, 512)],
                            start=(ct == 0), stop=(ct == CT - 1))
                    tcos = rpool.tile([128, 512], BF16, tag="tcos")
                    tsin = rpool.tile([128, 512], BF16, tag="tsin")
                    tsw = rpool.tile([128, 512], BF16, tag="tsw")
                    nc.vector.tensor_mul(tcos, pq, cos_sb[:, ts(tq, 512)])
                    nc.vector.tensor_mul(tsin, pq, sin_sb[:, ts(tq, 512)])
                    nc.vector.stream_shuffle(tsw, tsin, SWAP16)
                    nc.gpsimd.tensor_tensor(
                        r8c[:], tcos, tsw, mybir.AluOpType.add)
                    # repack into the DoubleRow layout; alternate DMA queues
                    # (SP->HWDGE vs Pool->SWDGE) to spread trigger cost
                    sl = slice(512 * tq, 512 * (tq + 1))
                    for hp in range(2):
                        for u in range(2):
                            pb = 64 * hp + 32 * u
                            nc.sync.dma_start(
                                qk8[32 * hp:32 * hp + 32, ft, u, sl],
                                r8c[pb:pb + 32, :])

                def v_tile(tt):
                    pv = psum.tile([128, 512], F32, tag="pq", bufs=2, name="pv")
                    for ct in range(CT):
                        nc.tensor.matmul(
                            pv,
                            lhsT=xT_sb[:, ct, ts(tt, 128)],
                            rhs=wv_sb[:, ct, :],
                            start=(ct == 0), stop=(ct == CT - 1))
                    nc.scalar.activation(
                        v16[:, tt, :, 0:64],
                        pv.rearrange("p (h d) -> p h d", h=HG),
                        mybir.ActivationFunctionType.Copy, scale=1.0)

                # Deferred-work queue: attention part2 (attn@v, norm,
                # transpose) and projection tiles are queued as closures and
                # drained between the NEXT head's score/exp chunks, so the
                # in-order PE stream always has ready work while exp lags.
                p2q = []

                def drain(n):
                    for _ in range(n):
                        if p2q:
                            p2q.pop(0)()

                def drain_all():
                    while p2q:
                        p2q.pop(0)()

                def attention_part1(h, it, inject=None, dve=None):
                    """Scores + exp for (h, it); returns px tiles."""
                    dve = dve or exp_dve
                    hp, p2 = h % 2, h // 2
                    qslot, kslot = p2, 4 + p2
                    base = 32 * hp
                    pxs = []
                    for jt in range(NJT):
                        sp = psum.tile([128, 512], F32, tag="sp", bufs=3,
                                       name="sp")
                        nc.tensor.matmul(
                            sp,
                            lhsT=qk8[base:base + 32, kslot, :, ts(jt, 128)],
                            rhs=qk8[base:base + 32, qslot, :, ts(it, 512)],
                            start=True, stop=True, perf_mode=DR)
                        px = pxpool.tile([128, 512], BF16, tag="px")
                        pxs.append(px)
                        if (exp_cnt[0] * dve[0]) % dve[1] < dve[0]:
                            nc.vector.tensor_scalar(
                                px.bitcast(I16), sp, SCHR_A, SCHR_B,
                                mybir.AluOpType.mult, mybir.AluOpType.add)
                        else:
                            nc.scalar.activation(
                                px, sp, mybir.ActivationFunctionType.Exp,
                                scale=0.125)
                        exp_cnt[0] += 1
                        if debug and h == 0 and it == 0 and jt == 0:
                            dsp = opool.tile([128, 512], F32, tag="ot",
                                             name="dsp")
                            nc.vector.tensor_copy(dsp, sp)
                            nc.sync.dma_start(dbg_sp[:], dsp)
                            nc.sync.dma_start(dbg_px[:], px)
                        if inject is not None:
                            inject(jt)
                        drain(2 if jt % 4 == 3 else 1)
                    return pxs

                def queue_part2(h, it, pxs):
                    """attn@v + norm + transpose for (h, it), as closures."""
                    hp, p2 = h % 2, h // 2
                    state = {}

                    def alloc():
                        state["yu"] = psum.tile([128, 4, 65], F32, tag="yu",
                                                bufs=2, name="yu")

                    p2q.append(alloc)
                    # sequential accumulation groups per ic: a start resets
                    # the whole 2KB psum zero region, so ic groups must not
                    # interleave within the yu bank
                    for ic in range(4):
                        for jt0 in range(0, NJT, 4):
                            def avs(ic=ic, jt0=jt0):
                                for jt in range(jt0, jt0 + 4):
                                    nc.tensor.matmul(
                                        state["yu"][:, ic, :],
                                        lhsT=pxs[jt][:, ts(ic, 128)],
                                        rhs=v16[:, jt, h, 0:65],
                                        start=(jt == 0),
                                        stop=(jt == NJT - 1))
                            p2q.append(avs)

                    def norm():
                        yu = state["yu"]
                        rcp = npool.tile([128, 4], F32, tag="rcp")
                        nc.vector.reciprocal(rcp, yu[:, :, 64])
                        y2t = ypool.tile([128, 4, 64], BF16, tag="y2t")
                        for ic in range(4):
                            nc.vector.tensor_scalar(
                                y2t[:, ic, :], yu[:, ic, 0:64],
                                rcp[:, ic:ic + 1], None,
                                mybir.AluOpType.mult)
                        if debug and h == 0 and it == 0:
                            dyu = opool.tile([128, 4, 65], F32, tag="dyu",
                                             name="dyu")
                            nc.vector.tensor_copy(dyu, yu)
                            nc.sync.dma_start(dbg_yu[:], dyu)
                            nc.sync.dma_start(dbg_y2t[:], y2t)
                        pt = psum.tile([64, 4, 128], BF16, tag="tr", bufs=1,
                                       name="pt")
                        for ic in range(4):
                            nc.tensor.transpose(pt[:, ic, :], y2t[:, ic, :],
                                                id_sb[:])
                        nc.scalar.activation(
                            y2_sb[64 * hp:64 * hp + 64, p2, ts(it, 512)],
                            pt[:].rearrange("p a t -> p (a t)"),
                            mybir.ActivationFunctionType.Copy, scale=1.0)

                    p2q.append(norm)

                def queue_proj(it):
                    # output projection for token chunk `it` (all heads done)
                    for tt in range(4 * it, 4 * it + 4):
                        for on in range(2):
                            def proj(tt=tt, on=on):
                                po = psum.tile([128, 512], F32, tag="pq",
                                               bufs=2, name="po")
                                for d4 in range(DV // 128):
                                    nc.tensor.matmul(
                                        po,
                                        lhsT=y2_sb[:, d4, ts(tt, 128)],
                                        rhs=wp_sb[:, d4, ts(on, 512)],
                                        start=(d4 == 0),
                                        stop=(d4 == DV // 128 - 1))
                                ot = opool.tile([128, 512], F32, tag="ot")
                                nc.scalar.activation(
                                    ot, po,
                                    mybir.ActivationFunctionType.Copy,
                                    scale=1.0)
                                nc.sync.dma_start(
                                    out_d[ts(tt, 128), ts(on, 512)], ot)
                            p2q.append(proj)

                # ---------------- emission schedule ----------------
                # deadline-sorted queue of remaining q/k projection tiles:
                # k tiles of pair p and q(p, it) must exist before
                # part1(h=2p, it). Tiles are popped opportunistically (1 per
                # 4 chunks) so projection PE work fills softmax bubbles.
                sched = []
                for p in range(4):
                    for tq in range(NIT):
                        if p > 0:
                            sched.append(((0, 2 * p), 4 + p, tq))  # k(p)
                        if not (p == 0 and tq == 0):
                            sched.append(((tq, 2 * p), p, tq))     # q(p,tq)
                sched.sort(key=lambda e: e[0])

                def sched_pop():
                    if sched:
                        _, ft, tq = sched.pop(0)
                        qk_tile(ft, tq)

                # startup: k of pair 0 (all chunks) + q(pair0, it0) + v(0)
                for tq in range(NIT):
                    qk_tile(4, tq)
                qk_tile(0, 0)
                v_tile(0)

                def inject00(jt):
                    # all v tiles stream during h0: part2(h0)'s avs (drained
                    # during h1) must be emitted after the v16 writes they read
                    if jt < NJT - 1:
                        v_tile(jt + 1)

                def inject_sched(jt):
                    if jt % 4 == 1:
                        sched_pop()

                for it in range(NIT):
                    for h in range(HG):
                        # force-emit overdue projection tiles
                        while sched and sched[0][0] <= (it, h):
                            sched_pop()
                        inj = inject00 if (it == 0 and h == 0) else inject_sched
                        dve = (1, 3) if it == 0 else (4, 9)
                        pxs = attention_part1(h, it, inject=inj, dve=dve)
                        queue_part2(h, it, pxs)
                    queue_proj(it)
                drain_all()
                if debug:
                    nc.sync.dma_start(dbg_qk8[:], qk8[:])
                    nc.sync.dma_start(dbg_v16[:], v16[:])
                    nc.sync.dma_start(dbg_y2[:], y2_sb[:])

    nc.finalize()
    return nc


_NC_CACHE = None


def _get_nc():
    global _NC_CACHE
    if _NC_CACHE is None:
        _NC_CACHE = build_nc()
    return _NC_CACHE


# ---------------------------------------------------------------- entry point

def kernel(x, freqs, W_qkv, W_proj, b_proj, _trace=False):
    x = np.asarray(x, dtype=np.float32)
    freqs = np.asarray(freqs, dtype=np.float32)
    W_qkv = np.asarray(W_qkv, dtype=np.float32)
    W_proj = np.asarray(W_proj, dtype=np.float32)
    b_proj = np.asarray(b_proj, dtype=np.float32)

    perm = _perm64()
    cosR, sinX = _cos_sin_tiles(freqs)
    ident = np.eye(128, dtype=np.float32).astype(ml_dtypes.bfloat16)

    wqkT = {}
    wvT = {}
    wpT = {}
    for g in range(G):
        rows = []
        for blk in (0, 1):  # q rows then k rows
            for hh in range(HG):
                h = g * HG + hh
                base = blk * DIM + h * D
                rows.append(W_qkv[base + perm])
        wqkT[g] = np.ascontiguousarray(
            np.concatenate(rows, axis=0).T).astype(ml_dtypes.bfloat16)
        wvT[g] = np.ascontiguousarray(
            W_qkv[2 * DIM + g * DV: 2 * DIM + (g + 1) * DV].T
        ).astype(ml_dtypes.bfloat16)
        wpT[g] = np.ascontiguousarray(
            W_proj[:, g * DV:(g + 1) * DV].T).astype(ml_dtypes.bfloat16)

    xT_b = {b: np.ascontiguousarray(x[b].T).astype(ml_dtypes.bfloat16)
            for b in range(B)}
    in_maps = []
    for core in range(N_CORES):
        b, g = core // G, core % G
        in_maps.append({
            "xT": xT_b[b],
            "wqkT": wqkT[g],
            "wvT": wvT[g],
            "wpT": wpT[g],
            "cosR": cosR,
            "sinX": sinX,
            "ident": ident,
        })

    from concourse import bass_utils

    nc = _get_nc()
    res = bass_utils.run_bass_kernel_spmd(
        nc, in_maps, core_ids=list(range(N_CORES)), trace=_trace)

    out = np.zeros((B, T, DIM), dtype=np.float32)
    for core in range(N_CORES):
        b = core // G
        out[b] += res.results[core]["out_part"]
    out += b_proj
    if _trace:
        return out, res
    return out
